# revision 1
# baseline (speedup 1.0000x reference)
"""ConvDVSGestureSNN Trainium2 kernel: 8-core data-parallel (16 batch each).

Per core: conv1 folded with BN+pool into a 6x6 stride-2 conv (fp32r matmuls,
6 x-shift replicas of x DMA'd per timestep); LIF1; conv2 as 6x6 stride-2 over
spike-complement (bf16, spikes exact in bf16); LIF2; fc1 (bf16); adaptive LIF;
fc_out (fp32); leaky output accumulator. T=50 loop fully unrolled.
"""
import numpy as np

B_LOC, T = 16, 50
N_FC, N_OUT = 256, 11
EPS = 1e-5


def _sig(z):
    return 1.0 / (1.0 + np.exp(-np.asarray(z, np.float64)))


def _build_nc():
    import concourse.bass as bass
    import concourse.mybir as mybir
    import concourse.tile as tile
    from concourse import bacc
    from concourse.masks import make_identity

    dt = mybir.dt
    Alu = mybir.AluOpType
    Act = mybir.ActivationFunctionType

    nc = bacc.Bacc("TRN2", target_bir_lowering=False, debug=False)

    xr = nc.dram_tensor("xr", [B_LOC, T, 2056], dt.float32r, kind="ExternalInput")
    A1 = nc.dram_tensor("A1", [12, 6 * 32], dt.float32r, kind="ExternalInput")
    A2 = nc.dram_tensor("A2", [32, 36 * 64], dt.bfloat16, kind="ExternalInput")
    F1T = nc.dram_tensor("F1T", [64, 25 * 256], dt.bfloat16, kind="ExternalInput")
    FO = nc.dram_tensor("FO", [128, 2 * N_OUT], dt.float32, kind="ExternalInput")
    B1C = nc.dram_tensor("B1C", [32, 1], dt.float32, kind="ExternalInput")
    B2C = nc.dram_tensor("B2C", [64, 1], dt.float32, kind="ExternalInput")
    BETA1 = nc.dram_tensor("BETA1", [128, 1], dt.float32, kind="ExternalInput")
    BETA2 = nc.dram_tensor("BETA2", [64, 1], dt.float32, kind="ExternalInput")
    FCP = nc.dram_tensor("FCP", [16, 4 * 256], dt.float32, kind="ExternalInput")
    OUT = nc.dram_tensor("out", [B_LOC, N_OUT], dt.float32, kind="ExternalOutput")

    with tile.TileContext(nc) as tc:
        with tc.tile_pool(name="const", bufs=1) as cp, \
             tc.tile_pool(name="state", bufs=1) as st, \
             tc.tile_pool(name="x6", bufs=2) as xp, \
             tc.tile_pool(name="work", bufs=2) as wp, \
             tc.tile_pool(name="ps1", bufs=2, space="PSUM") as ps1, \
             tc.tile_pool(name="ps2", bufs=1, space="PSUM") as ps2, \
             tc.tile_pool(name="psf", bufs=1, space="PSUM") as psf, \
             tc.tile_pool(name="pst", bufs=2, space="PSUM") as pst, \
             tc.tile_pool(name="pso", bufs=1, space="PSUM") as pso:

            a1 = cp.tile([12, 6 * 32], dt.float32r)
            a2 = cp.tile([32, 36 * 64], dt.bfloat16)
            f1t = cp.tile([64, 25 * 256], dt.bfloat16)
            fo = cp.tile([128, 2 * N_OUT], dt.float32)
            b1c = cp.tile([32, 1], dt.float32)
            b2c = cp.tile([64, 1], dt.float32)
            beta1 = cp.tile([128, 1], dt.float32)
            beta2 = cp.tile([64, 1], dt.float32)
            fcp = cp.tile([16, 4 * 256], dt.float32)
            ident = cp.tile([128, 128], dt.float32)
            for dst, src in ((a1, A1), (a2, A2), (f1t, F1T), (fo, FO), (b1c, B1C),
                             (b2c, B2C), (beta1, BETA1), (beta2, BETA2), (fcp, FCP)):
                nc.sync.dma_start(dst[:], src[:])
            make_identity(nc, ident[:])

            # persistent state
            v1 = st.tile([128, 784], dt.float32)      # rows (bgroup4, co32), free (4b, 196)
            v2 = st.tile([64, 400], dt.float32)       # rows co, free (16b, 25)
            vfc = st.tile([16, 256], dt.float32)
            afc = st.tile([16, 256], dt.float32)
            spkfc = st.tile([16, 256], dt.float32)
            vo = st.tile([16, N_OUT], dt.float32)
            acc = st.tile([16, N_OUT], dt.float32)
            for z in (v1, v2, vfc, afc, spkfc, vo, acc):
                nc.gpsimd.memset(z[:], 0.0)

            alpha16 = fcp[:, 0:256]
            rho16 = fcp[:, 256:512]
            rhoc16 = fcp[:, 512:768]
            ba16 = fcp[:, 768:1024]

            for t in range(T):
                # ---- load 6 x-shift replicas of x_t: rows (j*2+ci) = x[b, y, j..j+26]
                x6 = xp.tile([12, 16 * 1024], dt.float32r)
                for j in range(6):
                    for ci in range(2):
                        src = bass.AP(xr[:].tensor, t * 2056 + ci * 1024 + j,
                                      [[1, 1], [102800, 16], [1, 1024]])
                        nc.sync.dma_start(
                            x6[j * 2 + ci:j * 2 + ci + 1, :]
                            .rearrange("p (b f) -> p b f", b=16, f=1024), src)
                x6v = x6[:].rearrange("p (b y x) -> p b y x", b=16, y=32, x=32)

                # ---- conv1 (6x6 stride2) -> c1full [128,(4b,196)], 8 chunks of 2 batches
                c1full = wp.tile([128, 784], dt.float32, tag="c1full")
                for c in range(8):
                    p1 = ps1.tile([32, 392], dt.float32, tag="p1")
                    for ey in range(6):
                        rhs = x6v[0:12, 2 * c:2 * c + 2, ey:ey + 27:2, 0:27:2]
                        nc.tensor.matmul(
                            p1[:].rearrange("p (b y x) -> p b y x", b=2, y=14, x=14),
                            a1[:, ey * 32:(ey + 1) * 32], rhs,
                            start=(ey == 0), stop=(ey == 5))
                    # evac + bias1 into v1-layout: rows 32*(c//2), free (c%2)*392
                    nc.scalar.activation(
                        c1full[32 * (c // 2):32 * (c // 2) + 32,
                               (c % 2) * 392:(c % 2) * 392 + 392],
                        p1[:], Act.Identity, bias=b1c[:])

                # ---- LIF1 on [128, 784]
                nc.vector.tensor_scalar(v1[:], v1[:], beta1[:], None, Alu.mult)
                nc.vector.tensor_tensor(v1[:], v1[:], c1full[:], Alu.add)
                spk1inv = wp.tile([32, 3136], dt.bfloat16, tag="spk1")
                for g in range(4):
                    nc.vector.tensor_scalar(spk1inv[:, g * 784:(g + 1) * 784],
                                            v1[32 * g:32 * g + 32, :], 1.0, None,
                                            Alu.is_le)
                m1 = wp.tile([128, 784], dt.float32, tag="m1")
                for g in range(4):
                    nc.vector.tensor_copy(m1[32 * g:32 * g + 32, :],
                                          spk1inv[:, g * 784:(g + 1) * 784])
                nc.vector.tensor_tensor(v1[:], v1[:], m1[:], Alu.mult)

                # ---- conv2 (6x6 stride2 over complement, bf16) -> psum [64,(16b,25)]
                s1v = spk1inv[:].rearrange("p (b y x) -> p b y x", b=16, y=14, x=14)
                p2 = ps2.tile([64, 400], dt.float32, tag="p2")
                k = 0
                for ey in range(6):
                    for ex in range(6):
                        rhs = s1v[:, :, ey:ey + 9:2, ex:ex + 9:2]
                        nc.tensor.matmul(
                            p2[:].rearrange("p (b y x) -> p b y x", b=16, y=5, x=5),
                            a2[:, k * 64:(k + 1) * 64], rhs,
                            start=(k == 0), stop=(k == 35))
                        k += 1
                c2s = wp.tile([64, 400], dt.float32, tag="c2s")
                nc.scalar.activation(c2s[:], p2[:], Act.Identity, bias=b2c[:])

                # ---- LIF2 on [64, 400]
                nc.vector.tensor_scalar(v2[:], v2[:], beta2[:], None, Alu.mult)
                nc.vector.tensor_tensor(v2[:], v2[:], c2s[:], Alu.add)
                spk2 = wp.tile([64, 400], dt.bfloat16, tag="spk2")
                nc.vector.tensor_scalar(spk2[:], v2[:], 1.0, None, Alu.is_gt)
                m2 = wp.tile([64, 400], dt.float32, tag="m2")
                nc.vector.tensor_scalar(m2[:], v2[:], 1.0, None, Alu.is_le)
                nc.vector.tensor_tensor(v2[:], v2[:], m2[:], Alu.mult)

                # ---- fc1: I_fc [16b, 256] = sum_s spk2[:, (b,s)].T @ f1t_s
                pf = psf.tile([16, 256], dt.float32, tag="pf")
                for s in range(25):
                    nc.tensor.matmul(pf[:], spk2[:, s::25],
                                     f1t[:, s * 256:(s + 1) * 256],
                                     start=(s == 0), stop=(s == 24))

                # ---- adaptive LIF (order: a-update w/ prev spk, v-update, spike)
                nc.vector.tensor_tensor(afc[:], afc[:], rho16, Alu.mult)
                tmp = wp.tile([16, 256], dt.float32, tag="tmp")
                nc.vector.tensor_tensor(tmp[:], rhoc16, spkfc[:], Alu.mult)
                nc.vector.tensor_tensor(afc[:], afc[:], tmp[:], Alu.add)
                nc.vector.tensor_tensor(vfc[:], vfc[:], alpha16, Alu.mult)
                nc.vector.tensor_tensor(vfc[:], vfc[:], pf[:], Alu.add)
                th = wp.tile([16, 256], dt.float32, tag="th")
                nc.vector.tensor_tensor(th[:], ba16, afc[:], Alu.mult)
                nc.vector.tensor_scalar(th[:], th[:], 1.0, None, Alu.add)
                nc.vector.tensor_tensor(spkfc[:], vfc[:], th[:], Alu.is_gt)
                mf = wp.tile([16, 256], dt.float32, tag="mf")
                nc.vector.tensor_tensor(mf[:], vfc[:], th[:], Alu.is_le)
                nc.vector.tensor_tensor(vfc[:], vfc[:], mf[:], Alu.mult)

                # ---- fc_out: transpose spkfc chunks, 2 matmuls -> psum [16,11]
                po = pso.tile([16, N_OUT], dt.float32, tag="po")
                for kk in range(2):
                    ptr = pst.tile([128, 16], dt.float32, tag="ptr")
                    nc.tensor.transpose(ptr[:], spkfc[:, kk * 128:(kk + 1) * 128],
                                        ident[0:16, 0:16])
                    str_ = wp.tile([128, 16], dt.float32, tag="str")
                    nc.vector.tensor_copy(str_[:], ptr[:])
                    nc.tensor.matmul(po[:], str_[:],
                                     fo[:, kk * N_OUT:(kk + 1) * N_OUT],
                                     start=(kk == 0), stop=(kk == 1))

                nc.vector.tensor_scalar(vo[:], vo[:], float(_BO[0]), None, Alu.mult)
                nc.vector.tensor_tensor(vo[:], vo[:], po[:], Alu.add)
                nc.vector.tensor_tensor(acc[:], acc[:], vo[:], Alu.add)

            nc.sync.dma_start(OUT[:], acc[:])

    nc.compile()
    return nc


_BO = [0.0]  # set before _build_nc
_NC_CACHE = None


def _prep(inputs):
    """Host-side folding of BN/pool/decay constants into weights."""
    f64 = lambda a: np.asarray(a, np.float64)
    s1 = f64(inputs["bn1_gamma"]) / np.sqrt(f64(inputs["bn1_var"]) + EPS)
    sh1 = f64(inputs["bn1_beta"]) - f64(inputs["bn1_mean"]) * s1
    s2 = f64(inputs["bn2_gamma"]) / np.sqrt(f64(inputs["bn2_var"]) + EPS)
    sh2 = f64(inputs["bn2_beta"]) - f64(inputs["bn2_mean"]) * s2
    b1 = _sig(inputs["beta_conv1_raw"])
    b2 = _sig(inputs["beta_conv2_raw"])
    alpha = _sig(inputs["alpha_raw"])
    rho = _sig(inputs["rho_raw"])
    bo = float(_sig(inputs["beta_out"]))

    w1 = f64(inputs["conv1_w"])  # (32,2,5,5)
    w2 = f64(inputs["conv2_w"])  # (64,32,5,5)
    # fold 2x2 mean-pool: 6x6 stride-2 effective kernels, scaled
    w1e = np.zeros((32, 2, 6, 6))
    w2e = np.zeros((64, 32, 6, 6))
    for dy in range(5):
        for dx in range(5):
            for p in range(2):
                for q in range(2):
                    w1e[:, :, dy + p, dx + q] += 0.25 * w1[:, :, dy, dx]
                    w2e[:, :, dy + p, dx + q] += 0.25 * w2[:, :, dy, dx]
    w1e *= (s1 * (1 - b1))[:, None, None, None]
    w2e *= (s2 * (1 - b2))[:, None, None, None]

    A1 = np.zeros((12, 6 * 32), np.float32)
    for j in range(6):
        for ci in range(2):
            for ey in range(6):
                A1[j * 2 + ci, ey * 32:(ey + 1) * 32] = w1e[:, ci, ey, j]
    A2 = np.zeros((32, 36 * 64), np.float32)
    for ey in range(6):
        for ex in range(6):
            A2[:, (ey * 6 + ex) * 64:(ey * 6 + ex + 1) * 64] = -w2e[:, :, ey, ex].T
    c2const = w2e.sum(axis=(1, 2, 3))  # conv2 of all-ones input
    B1C = ((1 - b1) * sh1).astype(np.float32).reshape(32, 1)
    B2C = ((1 - b2) * sh2 + c2const).astype(np.float32).reshape(64, 1)
    BETA1 = np.tile(b1.astype(np.float32), 4).reshape(128, 1)
    BETA2 = b2.astype(np.float32).reshape(64, 1)

    f1 = f64(inputs["fc1_w"]) * (1 - alpha)[:, None]  # (256,1600)
    F1T = np.zeros((64, 25 * 256), np.float32)
    for s in range(25):
        F1T[:, s * 256:(s + 1) * 256] = f1[:, np.arange(64) * 25 + s].T
    FO = np.zeros((128, 2 * N_OUT), np.float32)
    foW = f64(inputs["fc_out_w"]) * (1 - bo) / T  # (11,256)
    FO[:, 0:N_OUT] = foW[:, 0:128].T
    FO[:, N_OUT:2 * N_OUT] = foW[:, 128:256].T
    FCP = np.zeros((16, 4 * 256), np.float32)
    FCP[:, 0:256] = alpha[None, :]
    FCP[:, 256:512] = rho[None, :]
    FCP[:, 512:768] = (1 - rho)[None, :]
    FCP[:, 768:1024] = f64(inputs["beta_a"])[None, :]

    import ml_dtypes
    return dict(A1=A1, A2=A2.astype(ml_dtypes.bfloat16),
                F1T=F1T.astype(ml_dtypes.bfloat16), FO=FO, B1C=B1C, B2C=B2C,
                BETA1=BETA1, BETA2=BETA2, FCP=FCP), bo


def _run(inputs, trace=False):
    global _NC_CACHE
    from concourse.bass_utils import run_bass_kernel_spmd
    aux, bo = _prep(inputs)
    _BO[0] = bo
    if _NC_CACHE is None:
        _NC_CACHE = _build_nc()
    nc = _NC_CACHE
    x = np.ascontiguousarray(np.asarray(inputs["x"], np.float32))
    in_maps = []
    for c in range(8):
        m = dict(aux)
        xc = x[c * B_LOC:(c + 1) * B_LOC].reshape(B_LOC, T, 2048)
        xp_ = np.zeros((B_LOC, T, 2056), np.float32)
        xp_[:, :, :2048] = xc
        m["xr"] = xp_
        in_maps.append(m)
    res = run_bass_kernel_spmd(nc, in_maps, core_ids=list(range(8)), trace=trace)
    out = np.concatenate([res.results[c]["out"] for c in range(8)], axis=0)
    return out.astype(np.float32), res


def kernel(**inputs) -> np.ndarray:
    out, _ = _run(inputs, trace=False)
    return out

